# revision 14
# baseline (speedup 1.0000x reference)
"""Trainium2 Bass kernel for the pixel-RNN (tanh RNN, T=784, H=512, B=256).

Data-parallel over batch: 32 samples per core on 8 cores, fp16 matmul
operands (verified against the fp32 reference: max logit perturbation
~0.001 vs min decision margin 0.0031; loss rel err ~5e-6; the `correct`
count is unchanged).

Core idea: keep the recurrent state ONLY in transposed layout
hT [H on partitions, batch on free] and compute the recurrence in that
orientation:  hT_new[j, b] = tanh(sum_k W^T[k, j-chunk] @ hT[k, b] + x).
The stationary operand is then a W^T chunk [128, 128] fp16, whose
LDWEIGHTS takes the Fast-Weight-Load path (~25 ns, needs a full
128x128 non-fp32 stationary); the N=32 matmuls are issue-floor bound
(~27 ns). No transposes and no PSUM->SBUF copies: ScalarE tanh reads
PSUM and writes hT (SBUF, fp16) directly, so the only cross-engine
chain per step is matmul -> tanh -> matmul.

Per step (one PSUM bank per parity):
  - 16 recurrence matmuls, ordered [all-c x (k0,k1)] then, c-major,
    [c x (k2,k3)]: psum regions c0,c1 complete at slot 12, so tanh_c01
    (which gates the next step's k01 matmuls) starts as early as
    possible while tanh_c23's latency hides under the following slots.
  - 4 x-term matmuls (K=2: rows [x_t; 1] against [w_ih; b] columns)
    emitted for step t+1 after step t's recurrence; the first opens the
    bank's accumulation group (start=True), later region writers set
    their own has_written bits, the recurrence accumulates on top.
  - 2 tanh halves [128, 64] on ScalarE.
The step period (~1.17 us) is bound by the dependency cycle
c01-completion -> sem -> tanh -> sem -> next step's matmuls; the PE
engine pipeline costs ~130 ns on the first matmul after each semaphore
wait, which is included in that cycle.

Final linear head (10 classes) on device; log-softmax / loss / argmax
on host (tiny [256,10] reduction).
"""

import sys

if "/opt/trn_rl_repo" not in sys.path:
    sys.path.insert(0, "/opt/trn_rl_repo")

import numpy as np

B, T, H, NCLS = 256, 784, 512, 10
NCORES = 8
BC = B // NCORES   # 32 samples per core
KC = H // 128      # 4 chunks of the hidden dim

_BUILD_CACHE = {}


def _build(t_steps=T, split_waits=True):
    """Build the Bass module (single program, run SPMD on 8 cores)."""
    import concourse.bass as bass
    import concourse.mybir as mybir
    from concourse import tile

    f16 = mybir.dt.float16
    f32 = mybir.dt.float32
    Tanh = mybir.ActivationFunctionType.Tanh

    nc = bass.Bass(
        "TRN2",
        target_bir_lowering=False,
        debug=False,
        enable_asserts=False,
        num_devices=NCORES,
    )

    d_xT = nc.dram_tensor("xT", (2, t_steps * BC), f16, kind="ExternalInput").ap()
    d_w2b = nc.dram_tensor("w2b", (2, H), f16, kind="ExternalInput").ap()
    d_WT2 = nc.dram_tensor("WT2", (128, 16 * 128), f16, kind="ExternalInput").ap()
    d_lWT = nc.dram_tensor("lWT", (128, KC * NCLS), f16, kind="ExternalInput").ap()
    d_out = nc.dram_tensor("logitsT", (NCLS, BC), f32, kind="ExternalOutput").ap()

    with tile.TileContext(nc) as tc:
        with (
            tc.tile_pool(name="const", bufs=1) as cpool,
            tc.tile_pool(name="ps", bufs=1, space="PSUM") as ppool,
        ):
            xT = cpool.tile([2, t_steps * BC], f16, tag="xT")
            w2b = cpool.tile([2, H], f16, tag="w2b")
            WT2 = cpool.tile([128, 16 * 128], f16, tag="WT2")
            lWT = cpool.tile([128, KC * NCLS], f16, tag="lWT")
            out_sb = cpool.tile([NCLS, BC], f32, tag="out")

            # hT layout: [128 partitions = j within chunk, (k-chunk, b)],
            # ping-pong parity
            hT = [cpool.tile([128, KC, BC], f16, tag=f"hT{p}",
                             name=f"hT{p}") for p in range(2)]
            # PSUM: full bank per parity (cols 0:128 used) + dummy scratch
            ps = [ppool.tile([128, 512], f32, tag=f"ps{p}", name=f"ps{p}")
                  for p in range(2)]
            scr = ppool.tile([128, 512], f32, tag="scr", name="scr")

            for i in range(8):
                nc.sync.dma_start(
                    out=WT2[:, i * 256:(i + 1) * 256],
                    in_=d_WT2[:, i * 256:(i + 1) * 256],
                )
            nc.sync.dma_start(out=xT[:, :], in_=d_xT)
            nc.sync.dma_start(out=w2b[:, :], in_=d_w2b)
            nc.sync.dma_start(out=lWT[:, :], in_=d_lWT)

            # gate matmuls: one per DMA, absorbing that queue's semaphore
            # into the PE's observed clock (results discarded)
            gates = [
                (xT[0:2, 0:128], w2b[0:2, 0:64]),
                (w2b[0:2, 0:128], xT[0:2, 0:64]),
            ]
            for i in range(8):
                gates.append((WT2[:, i * 256:i * 256 + 128],
                              WT2[:, i * 256:i * 256 + 64]))
            # lWT's DMA semaphore rides on the end-of-loop head matmul
            # (multi-wait splitter handles the extra wait there).
            for glhs, grhs in gates:
                nc.tensor.matmul(scr[0:glhs.shape[-1], 0:grhs.shape[-1]],
                                 glhs, grhs, start=True, stop=True)

            # rec MM order: k01 for all c, then k23 for all c -> psum
            # regions c0,c1 complete at slot 12; tanh_c01 gates the next
            # step while tanh_c23 latency hides under the k23 slots.
            order = [(c, k) for k in (0, 1) for c in range(KC)] + \
                    [(c, k) for c in range(KC) for k in (2, 3)]

            def emit_x(t):
                # x-term + bias for step t: four K=2 matmuls into ps[t%2],
                # one per chunk region; the c0 one opens the bank's
                # accumulation group (start=True), later region writers
                # set their own has_written bits (start=False overwrites
                # untouched regions).
                pp_ = ps[t % 2]
                for c in range(KC):
                    nc.tensor.matmul(
                        pp_[:, c * BC:(c + 1) * BC],
                        w2b[0:2, c * 128:(c + 1) * 128],
                        xT[0:2, t * BC:(t + 1) * BC],
                        start=(c == 0),
                        stop=(t == 0 and c == KC - 1),
                        skip_group_check=True,
                    )

            emit_x(0)
            for t in range(t_steps):
                p, q = t % 2, 1 - (t % 2)
                pg = ps[p]
                if t > 0:
                    for i, (c, k) in enumerate(order):
                        nc.tensor.matmul(
                            pg[:, c * BC:(c + 1) * BC],
                            WT2[:, (k * KC + c) * 128:(k * KC + c + 1) * 128],
                            hT[q][:, k, :],
                            start=False,
                            stop=(i == len(order) - 1),
                            skip_group_check=True,
                        )
                if t < t_steps - 1:
                    emit_x(t + 1)
                # tanh halves: PSUM f32 -> SBUF hT fp16
                nc.scalar.activation(hT[p][:, 0:2, :], pg[:, 0:64], Tanh)
                nc.scalar.activation(hT[p][:, 2:4, :], pg[:, 64:128], Tanh)

            # final linear head: logitsT[cls, b] = sum_j lin_W[cls, j] h[b, j]
            pl = (t_steps - 1) % 2
            pL = ps[1 - pl]
            for kc in range(KC):
                nc.tensor.matmul(
                    pL[0:NCLS, 0:BC],
                    lWT[:, kc * NCLS:(kc + 1) * NCLS],
                    hT[pl][:, kc, :],
                    start=(kc == 0),
                    stop=(kc == KC - 1),
                )
            nc.vector.tensor_copy(out_sb[:, :], pL[0:NCLS, 0:BC])
            nc.sync.dma_start(out=d_out, in_=out_sb[:, :])

    if split_waits:
        _split_multi_waits(nc, mybir)
    return nc


def _split_multi_waits(nc, mybir):
    """Walrus can pack only one sync wait into a HW instruction. Move any
    extra waits onto same-engine NoOps inserted right before (the engine's
    sequencer executes them in order, so semantics are unchanged)."""
    nid = 0
    for b in nc.m.functions[0].blocks:
        out = []
        changed = False
        for ins in b.instructions:
            si = getattr(ins, "sync_info", None)
            ws = list(getattr(si, "on_wait", []) or []) if si else []
            if len(ws) > 1:
                for w in ws[:-1]:
                    nid += 1
                    out.append(mybir.InstNoOp(
                        name=f"I-wsplit-{nid}",
                        engine=ins.engine,
                        sync_info=mybir.SyncInfo(on_wait=[w], on_update=[]),
                    ))
                ins.sync_info = mybir.SyncInfo(
                    on_wait=[ws[-1]], on_update=list(si.on_update or [])
                )
                changed = True
            out.append(ins)
        if changed:
            b.instructions = out
    return nc


def _pack_inputs(inputs, order, W_ih, b_ih, W_hh, b_hh, lin_W, t_steps=T):
    """Host-side shard packing: returns in_maps list (one dict per core)."""
    x = np.asarray(inputs, np.float32)[:, np.asarray(order, np.int64)]
    x = np.ascontiguousarray(x[:, :t_steps]).astype(np.float16)  # [B, T]

    W = np.asarray(W_hh, np.float32)
    WT2 = np.zeros((128, 16 * 128), np.float16)
    for k in range(KC):
        for c in range(KC):
            WT2[:, (k * KC + c) * 128:(k * KC + c + 1) * 128] = \
                W[c * 128:(c + 1) * 128, k * 128:(k + 1) * 128].T

    wv = np.asarray(W_ih, np.float32)[:, 0]
    bv = np.asarray(b_ih, np.float32) + np.asarray(b_hh, np.float32)
    w2b = np.stack([wv, bv]).astype(np.float16)  # [2, H]

    lWT = np.ascontiguousarray(
        np.asarray(lin_W, np.float32).T.reshape(KC, 128, NCLS)
        .transpose(1, 0, 2).reshape(128, KC * NCLS)
    ).astype(np.float16)

    in_maps = []
    for core in range(NCORES):
        xc = x[core * BC:(core + 1) * BC]  # [32, T] fp16
        xTv = np.ones((2, t_steps * BC), np.float16)
        xTv[0] = xc.T.reshape(-1)
        in_maps.append(
            {"xT": xTv, "w2b": w2b, "WT2": WT2, "lWT": lWT}
        )
    return in_maps


def _run(inputs, y, order, W_ih, b_ih, W_hh, b_hh, lin_W, lin_b, trace=False):
    from concourse import bass_utils

    key = T
    if key not in _BUILD_CACHE:
        _BUILD_CACHE[key] = _build(T)
    nc = _BUILD_CACHE[key]

    in_maps = _pack_inputs(inputs, order, W_ih, b_ih, W_hh, b_hh, lin_W, T)
    res = bass_utils.run_bass_kernel_spmd(
        nc, in_maps, core_ids=list(range(NCORES)), trace=trace
    )

    logits = np.empty((B, NCLS), np.float32)
    for c in range(NCORES):
        logits[c * BC:(c + 1) * BC] = res.results[c]["logitsT"].T
    logits = logits + np.asarray(lin_b, np.float32)[None, :]

    yv = np.asarray(y).astype(np.int64)
    m = logits.max(axis=1, keepdims=True)
    logp = logits - (np.log(np.exp(logits - m).sum(axis=1, keepdims=True)) + m)
    loss = np.float32(-logp[np.arange(B), yv].mean())
    correct = np.int32((logits.argmax(axis=1) == yv).sum())
    return (loss, correct), res


def kernel(inputs, y, order, W_ih, b_ih, W_hh, b_hh, lin_W, lin_b):
    out, _ = _run(inputs, y, order, W_ih, b_ih, W_hh, b_hh, lin_W, lin_b)
    return out
